# revision 23
# baseline (speedup 1.0000x reference)
"""Masked multi-head attention (B=8, N=1024, C=768, H=12) on 8 TRN2 NeuronCores.

Data-parallel: one batch element per core, no collectives.

Per-core layout strategy (everything feature-major / "transposed" so no
activations ever need transposing except x itself and v, both done by the
DMA XBAR transpose on bf16 data — the PE touches only real matmuls):
  xT   [C, N]    = dma-transpose of bf16(x)
  qkvT [3C, N]   = w_qkv.T @ x.T      (lhsT = w_qkv as stored)
  sT   [keys, q] = per-head k-slice @ qT; Dh=64 so a head PAIR packs into
                   the 128-row PE array (tile_position from base_partition),
                   and the two heads' matmuls are emitted back-to-back so
                   they stream through disjoint row groups concurrently
  p    = exp(sT*scale + maskbias)     mask folded into the per-partition
                                      activation bias (keys on partitions)
  avT  [Dh+1, q] = [v | 1].T @ p      ones column gives the softmax normalizer
  attnT[f, q]    = avT[0:64] * (1/avT[64]) broadcast via DRAM-bounce DMA
  out  [q, f']   = attnT-slice.T @ w_proj + b_proj (bias fused into the
                   PSUM->SBUF copy as a scalar_tensor_tensor add)

The attention inner loop is ACT(exp)-bound, and the PE queue is in-order,
so the next pair's qkv matmuls are software-pipelined INTO the current
pair's key-tile loop as PE filler behind the exp waits.

Matmuls run in bf16 (f32 PSUM accumulation): fp32/f32r matmuls stream
4-byte operands at ~2 cycles/row on TRN2 while bf16 streams 1/row.
Built on Bacc so matmul sync waits get legalized.
"""

import numpy as np
from contextlib import ExitStack

import concourse.bass as bass
import concourse.tile as tile
from concourse import bacc, mybir
from concourse.bass_utils import run_bass_kernel_spmd
from concourse.masks import make_identity

F32 = mybir.dt.float32
BF16 = mybir.dt.bfloat16
I32 = mybir.dt.int32
AF = mybir.ActivationFunctionType
ALU = mybir.AluOpType

B = 8
N = 1024          # tokens
C = 768           # channels
H = 12            # heads
DH = 64           # head dim
P = 128           # partitions
KT = C // P       # 6 contraction tiles over C
NPAIR = H // 2    # 6 head pairs (2 heads per 128-partition tile)
NT = N // P       # 8 token/key tiles
SCALE = DH ** -0.5
MASK_NEG = -60000.0
NCORES = 8


def _body(ctx, tc, x_ext, mask_ext, wqkv_ext, wproj_ext, bproj_ext, out_ext):
    nc = tc.nc

    singles = ctx.enter_context(tc.tile_pool(name="singles", bufs=1))
    xnat_pool = ctx.enter_context(tc.tile_pool(name="xnat", bufs=2))
    qkv_pool = ctx.enter_context(tc.tile_pool(name="qkv", bufs=3))
    pt_pool = ctx.enter_context(tc.tile_pool(name="pt", bufs=6))
    vext_pool = ctx.enter_context(tc.tile_pool(name="vext", bufs=4))
    vnat_pool = ctx.enter_context(tc.tile_pool(name="vnat", bufs=4))
    zb_pool = ctx.enter_context(tc.tile_pool(name="zb", bufs=2))
    ps_pool = ctx.enter_context(tc.tile_pool(name="ps", bufs=2, space="PSUM"))
    ps_av = ctx.enter_context(tc.tile_pool(name="ps_av", bufs=2, space="PSUM"))
    dram_pool = ctx.enter_context(tc.tile_pool(name="dram", bufs=2, space="DRAM"))

    # ---- constants ----
    maskb_i = singles.tile([P, NT], I32)
    nc.sync.dma_start(out=maskb_i[:], in_=mask_ext.rearrange("i p -> p i"))
    maskb_f = singles.tile([P, NT], F32)
    nc.vector.tensor_copy(out=maskb_f[:], in_=maskb_i[:])
    maskb = singles.tile([P, NT], F32)
    nc.vector.tensor_scalar_mul(maskb[:], maskb_f[:], MASK_NEG)

    # bias row broadcast to all partitions for the fused bias-add
    bias_bc = singles.tile([P, C], F32)
    nc.sync.dma_start(out=bias_bc[:], in_=bproj_ext[0:1, :].to_broadcast([P, C]))

    onesf = singles.tile([P, 2, 1], F32)
    nc.vector.memset(onesf[:], 1.0)

    ident_f = singles.tile([P, P], F32)
    make_identity(nc, ident_f[:])
    ident = singles.tile([P, P], BF16)
    nc.vector.tensor_copy(out=ident[:], in_=ident_f[:])

    # ---- weight preload + bf16 cast, chunked per contraction tile; the
    # casts run on GpSimd so the DVE queue stays clear for the x path ----
    wqkv_b = singles.tile([P, KT, 3 * C], BF16)
    wproj_sb = singles.tile([P, KT, C], BF16)
    wqkv_r = wqkv_ext.rearrange("(k p) n -> p k n", p=P)
    wstage = ctx.enter_context(tc.tile_pool(name="wstage", bufs=2))
    for k in range(KT):
        wq_f = wstage.tile([P, 3 * C], F32, tag="wq", name=f"wqf{k}")
        nc.sync.dma_start(out=wq_f[:], in_=wqkv_r[:, k, :])
        nc.gpsimd.tensor_copy(out=wqkv_b[:, k, :], in_=wq_f[:])

    # ---- x: DMA in (scalar queue, concurrent with the weight stream),
    # cast to bf16, PE-transpose into xT ----
    xT = singles.tile([P, KT, N], BF16)
    for t in range(NT):
        xt = xnat_pool.tile([P, C], F32, tag="xnat")
        nc.scalar.dma_start(out=xt[:], in_=x_ext[t * P:(t + 1) * P, :])
        xtb = xnat_pool.tile([P, C], BF16, tag="xnat_b", name=f"xtb{t}")
        nc.vector.tensor_copy(out=xtb[:], in_=xt[:])
        for k in range(KT):
            pst = ps_pool.tile([P, P], BF16, tag="ps", name=f"ps_x{t}_{k}")
            nc.tensor.transpose(
                out=pst[:], in_=xtb[:, k * P:(k + 1) * P], identity=ident[:])
            nc.vector.tensor_copy(out=xT[:, k, t * P:(t + 1) * P], in_=pst[:])

    wproj_f = wstage.tile([P, KT, C], F32, tag="wp")
    nc.gpsimd.dma_start(
        out=wproj_f[:], in_=wproj_ext.rearrange("(k p) n -> p k n", p=P))
    nc.gpsimd.tensor_copy(out=wproj_sb[:], in_=wproj_f[:])

    attnT = singles.tile([P, KT, N], BF16)

    # ---- qkv projection, split into 6 units per pair so units can be
    # software-pipelined into the previous pair's key-tile loop ----
    qkv_tiles = {}

    def qkv_unit(p, u):
        name, m = (("q", p), ("k", NPAIR + p), ("v", 2 * NPAIR + p))[u // 2]
        qc = u % 2
        if qc == 0:
            qkv_tiles[(p, name)] = qkv_pool.tile(
                [P, N], BF16, tag=name, name=f"{name}{p}")
        dst = qkv_tiles[(p, name)]
        ps = ps_pool.tile([P, 512], F32, tag="ps", name=f"ps_{name}{p}_{qc}")
        for k in range(KT):
            nc.tensor.matmul(
                out=ps[:],
                lhsT=wqkv_b[:, k, m * P:(m + 1) * P],
                rhs=xT[:, k, qc * 512:(qc + 1) * 512],
                start=(k == 0), stop=(k == KT - 1))
        nc.vector.tensor_copy(out=dst[:, qc * 512:(qc + 1) * 512], in_=ps[:])

    for u in range(6):
        qkv_unit(0, u)

    # ---- per head pair ----
    for p in range(NPAIR):
        qt = qkv_tiles[(p, "q")]
        kt_ = qkv_tiles[(p, "k")]
        vt = qkv_tiles[(p, "v")]

        av = [ps_av.tile([DH + 1, N], F32, tag="ps_av", name=f"av{p}_{hi}")
              for hi in range(2)]

        def pv_mms(ve_, pts_, kb_):
            for hi in range(2):
                for qc in range(2):
                    nc.tensor.matmul(
                        out=av[hi][:, qc * 512:(qc + 1) * 512],
                        lhsT=ve_[:, hi, :],
                        rhs=pts_[qc][:, 512 * hi:512 * (hi + 1)],
                        start=(kb_ == 0), stop=(kb_ == NT - 1))

        prev = None
        for kb in range(NT):
            # v natural block for this key tile
            vnat = ps_pool.tile([P, P], BF16, tag="ps", name=f"vn{p}_{kb}")
            nc.tensor.transpose(
                out=vnat[:], in_=vt[:, kb * P:(kb + 1) * P], identity=ident[:])
            ve = vext_pool.tile([P, 2, DH + 1], BF16, tag="ve", name=f"ve{p}_{kb}")
            nc.vector.tensor_copy(
                out=ve[:, :, 0:DH], in_=vnat[:].rearrange("p (h d) -> p h d", h=2))
            nc.vector.tensor_copy(out=ve[:, :, DH:DH + 1], in_=onesf[:])

            # both heads' score matmuls target one psum tile (h0 in the low
            # bank, h1 in the high bank) so they are consecutive on the PE
            # queue and stream through disjoint row groups concurrently;
            # the mask bias is per-key so one exp covers both heads
            pts = []
            for qc in range(2):
                psq = ps_pool.tile([P, N], F32, tag="ps", name=f"ps_s{p}_{kb}_{qc}")
                for hi in range(2):
                    nc.tensor.matmul(
                        out=psq[:, 512 * hi:512 * (hi + 1)],
                        lhsT=kt_[64 * hi:64 * (hi + 1), kb * P:(kb + 1) * P],
                        rhs=qt[64 * hi:64 * (hi + 1), qc * 512:(qc + 1) * 512],
                        start=True, stop=True)
                pt = pt_pool.tile([P, N], BF16, tag="pt", name=f"pt{p}_{kb}_{qc}")
                nc.scalar.activation(
                    out=pt[:], in_=psq[:], func=AF.Exp,
                    bias=maskb[:, kb:kb + 1], scale=SCALE)
                pts.append(pt)

            # p@v for the PREVIOUS key tile: its exps finished long ago, so
            # the in-order PE queue never waits on ACT here
            if prev is not None:
                pv_mms(*prev)

            # PE filler while this tile's exps run: next pair's qkv units
            if p + 1 < NPAIR and kb < 6:
                qkv_unit(p + 1, kb)

            prev = (ve, pts, kb)
        pv_mms(*prev)

        for hi in range(2):
            # copy out of PSUM promptly so the av slot frees for pair p+1
            av_sb = zb_pool.tile([DH + 1, N], F32, tag="av_sb", name=f"avs{p}_{hi}")
            nc.vector.tensor_copy(out=av_sb[:], in_=av[hi][:])
            zdram = dram_pool.tile([1, N], F32, tag="zdram", name=f"zd{p}_{hi}")
            nc.sync.dma_start(out=zdram[:], in_=av_sb[DH:DH + 1, :])
            zb = zb_pool.tile([DH, N], F32, tag="zb", name=f"zb{p}_{hi}")
            nc.sync.dma_start(out=zb[:], in_=zdram[0:1, :].to_broadcast([DH, N]))
            nc.vector.reciprocal_approx_fast(out=zb[:], in_=zb[:])
            nc.vector.scalar_tensor_tensor(
                out=attnT[64 * hi:64 * (hi + 1), p, :],
                in0=av_sb[0:DH, :], scalar=1.0, in1=zb[:],
                op0=ALU.mult, op1=ALU.mult)

    # ---- projection back to natural layout ----
    # psum-bank-aligned output chunks: [0:512) and [512:768)
    chunks = [(0, 512), (512, 256)]
    for m in range(NT):
        out_sb = xnat_pool.tile([P, C], F32, tag="out_sb", name=f"out_sb{m}")
        pps = [ps_pool.tile([P, w], F32, tag="ps", name=f"ps_proj{m}_{j}")
               for j, (lo, w) in enumerate(chunks)]
        for j, (lo, w) in enumerate(chunks):
            for k in range(KT):
                nc.tensor.matmul(
                    out=pps[j][:],
                    lhsT=attnT[:, k, m * P:(m + 1) * P],
                    rhs=wproj_sb[:, k, lo:lo + w],
                    start=(k == 0), stop=(k == KT - 1))
            nc.vector.scalar_tensor_tensor(
                out=out_sb[:, lo:lo + w], in0=pps[j][:], scalar=1.0,
                in1=bias_bc[:, lo:lo + w], op0=ALU.mult, op1=ALU.add)
        nc.sync.dma_start(out=out_ext[m * P:(m + 1) * P, :], in_=out_sb[:])


def build():
    nc = bacc.Bacc()
    x_ext = nc.declare_dram_parameter("x", [N, C], F32, isOutput=False)
    mask_ext = nc.declare_dram_parameter("mask", [NT, P], I32, isOutput=False)
    wqkv_ext = nc.declare_dram_parameter("w_qkv", [C, 3 * C], F32, isOutput=False)
    wproj_ext = nc.declare_dram_parameter("w_proj", [C, C], F32, isOutput=False)
    bproj_ext = nc.declare_dram_parameter("b_proj", [1, C], F32, isOutput=False)
    out_ext = nc.declare_dram_parameter("out", [N, C], F32, isOutput=True)

    with tile.TileContext(nc) as tc, ExitStack() as ctx:
        _body(ctx, tc, x_ext.ap(), mask_ext.ap(), wqkv_ext.ap(), wproj_ext.ap(),
              bproj_ext.ap(), out_ext.ap())
    nc.finalize()
    return nc


_NC_CACHE = None


def _get_nc():
    global _NC_CACHE
    if _NC_CACHE is None:
        _NC_CACHE = build()
    return _NC_CACHE


def _make_in_maps(inputs):
    x = np.ascontiguousarray(np.asarray(inputs["x"], dtype=np.float32))
    mask = np.ascontiguousarray(np.asarray(inputs["mask"], dtype=np.int32))
    w_qkv = np.ascontiguousarray(np.asarray(inputs["w_qkv"], dtype=np.float32))
    w_proj = np.ascontiguousarray(np.asarray(inputs["w_proj"], dtype=np.float32))
    b_proj = np.ascontiguousarray(
        np.asarray(inputs["b_proj"], dtype=np.float32)).reshape(1, C)
    return [
        {
            "x": x[b],
            "mask": mask[b].reshape(NT, P),
            "w_qkv": w_qkv,
            "w_proj": w_proj,
            "b_proj": b_proj,
        }
        for b in range(B)
    ]


def _run(inputs, trace=False, **kwargs):
    nc = _get_nc()
    in_maps = _make_in_maps(inputs)
    res = run_bass_kernel_spmd(nc, in_maps, list(range(NCORES)), trace=trace, **kwargs)
    out = np.stack([np.asarray(res.results[i]["out"]) for i in range(NCORES)])
    return out, res


def kernel(**inputs):
    out, _ = _run(inputs)
    return out


# revision 26
# speedup vs baseline: 1.0282x; 1.0282x over previous
"""Masked multi-head attention (B=8, N=1024, C=768, H=12) on 8 TRN2 NeuronCores.

Data-parallel: one batch element per core, no collectives.

Per-core layout strategy (everything feature-major / "transposed" so no
activations ever need transposing except x itself and v, both done by the
DMA XBAR transpose on bf16 data — the PE touches only real matmuls):
  xT   [C, N]    = dma-transpose of bf16(x)
  qkvT [3C, N]   = w_qkv.T @ x.T      (lhsT = w_qkv as stored)
  sT   [keys, q] = per-head k-slice @ qT; Dh=64 so a head PAIR packs into
                   the 128-row PE array (tile_position from base_partition),
                   and the two heads' matmuls are emitted back-to-back so
                   they stream through disjoint row groups concurrently
  p    = exp(sT*scale + maskbias)     mask folded into the per-partition
                                      activation bias (keys on partitions)
  avT  [Dh+1, q] = [v | 1].T @ p      ones column gives the softmax normalizer
  attnT[f, q]    = avT[0:64] * (1/avT[64]) broadcast via DRAM-bounce DMA
  out  [q, f']   = attnT-slice.T @ w_proj + b_proj (bias fused into the
                   PSUM->SBUF copy as a scalar_tensor_tensor add)

The attention inner loop is ACT(exp)-bound, and the PE queue is in-order,
so the next pair's qkv matmuls are software-pipelined INTO the current
pair's key-tile loop as PE filler behind the exp waits.

Matmuls run in bf16 (f32 PSUM accumulation): fp32/f32r matmuls stream
4-byte operands at ~2 cycles/row on TRN2 while bf16 streams 1/row.
Built on Bacc so matmul sync waits get legalized.
"""

import numpy as np
from contextlib import ExitStack

import concourse.bass as bass
import concourse.tile as tile
from concourse import bacc, mybir
from concourse.bass_utils import run_bass_kernel_spmd
from concourse.masks import make_identity

F32 = mybir.dt.float32
BF16 = mybir.dt.bfloat16
I32 = mybir.dt.int32
AF = mybir.ActivationFunctionType
ALU = mybir.AluOpType

B = 8
N = 1024          # tokens
C = 768           # channels
H = 12            # heads
DH = 64           # head dim
P = 128           # partitions
KT = C // P       # 6 contraction tiles over C
NPAIR = H // 2    # 6 head pairs (2 heads per 128-partition tile)
NT = N // P       # 8 token/key tiles
SCALE = DH ** -0.5
MASK_NEG = -60000.0
NCORES = 8


def _body(ctx, tc, x_ext, mask_ext, wqkv_ext, wproj_ext, bproj_ext, out_ext):
    nc = tc.nc

    singles = ctx.enter_context(tc.tile_pool(name="singles", bufs=1))
    xnat_pool = ctx.enter_context(tc.tile_pool(name="xnat", bufs=2))
    qkv_pool = ctx.enter_context(tc.tile_pool(name="qkv", bufs=3))
    pt_pool = ctx.enter_context(tc.tile_pool(name="pt", bufs=6))
    vext_pool = ctx.enter_context(tc.tile_pool(name="vext", bufs=4))
    vnat_pool = ctx.enter_context(tc.tile_pool(name="vnat", bufs=4))
    zb_pool = ctx.enter_context(tc.tile_pool(name="zb", bufs=2))
    ps_pool = ctx.enter_context(tc.tile_pool(name="ps", bufs=2, space="PSUM"))
    ps_av = ctx.enter_context(tc.tile_pool(name="ps_av", bufs=2, space="PSUM"))
    dram_pool = ctx.enter_context(tc.tile_pool(name="dram", bufs=2, space="DRAM"))

    # ---- constants ----
    maskb_i = singles.tile([P, NT], I32)
    nc.sync.dma_start(out=maskb_i[:], in_=mask_ext.rearrange("i p -> p i"))
    maskb_f = singles.tile([P, NT], F32)
    nc.vector.tensor_copy(out=maskb_f[:], in_=maskb_i[:])
    maskb = singles.tile([P, NT], F32)
    nc.vector.tensor_scalar_mul(maskb[:], maskb_f[:], MASK_NEG)

    # bias row broadcast to all partitions for the fused bias-add
    bias_bc = singles.tile([P, C], F32)
    nc.sync.dma_start(out=bias_bc[:], in_=bproj_ext[0:1, :].to_broadcast([P, C]))

    onesf = singles.tile([P, 2, 1], F32)
    nc.vector.memset(onesf[:], 1.0)

    ident_f = singles.tile([P, P], F32)
    make_identity(nc, ident_f[:])
    ident = singles.tile([P, P], BF16)
    nc.vector.tensor_copy(out=ident[:], in_=ident_f[:])

    # ---- weight preload + bf16 cast, chunked per contraction tile; the
    # casts run on GpSimd so the DVE queue stays clear for the x path ----
    wqkv_b = singles.tile([P, KT, 3 * C], BF16)
    wproj_sb = singles.tile([P, KT, C], BF16)
    wqkv_r = wqkv_ext.rearrange("(k p) n -> p k n", p=P)
    wstage = ctx.enter_context(tc.tile_pool(name="wstage", bufs=2))
    for k in range(KT):
        wq_f = wstage.tile([P, 3 * C], F32, tag="wq", name=f"wqf{k}")
        nc.sync.dma_start(out=wq_f[:], in_=wqkv_r[:, k, :])
        nc.scalar.activation(out=wqkv_b[:, k, :], in_=wq_f[:], func=AF.Copy)

    # ---- x: DMA in (scalar queue, concurrent with the weight stream),
    # cast to bf16, PE-transpose into xT ----
    xT = singles.tile([P, KT, N], BF16)
    for t in range(NT):
        xt = xnat_pool.tile([P, C], F32, tag="xnat")
        nc.scalar.dma_start(out=xt[:], in_=x_ext[t * P:(t + 1) * P, :])
        xtb = xnat_pool.tile([P, C], BF16, tag="xnat_b", name=f"xtb{t}")
        nc.vector.tensor_copy(out=xtb[:], in_=xt[:])
        for k in range(KT):
            pst = ps_pool.tile([P, P], BF16, tag="ps", name=f"ps_x{t}_{k}")
            nc.tensor.transpose(
                out=pst[:], in_=xtb[:, k * P:(k + 1) * P], identity=ident[:])
            nc.vector.tensor_copy(out=xT[:, k, t * P:(t + 1) * P], in_=pst[:])

    wproj_f = wstage.tile([P, KT, C], F32, tag="wp")
    nc.gpsimd.dma_start(
        out=wproj_f[:], in_=wproj_ext.rearrange("(k p) n -> p k n", p=P))
    nc.gpsimd.tensor_copy(out=wproj_sb[:], in_=wproj_f[:])

    attnT = singles.tile([P, KT, N], BF16)

    # persistent v_ext tiles: the ones column is written once and never
    # touched again; only the v part is rewritten per key tile
    ve_tiles = [singles.tile([P, 2, DH + 1], BF16, name=f"ve_s{i}")
                for i in range(4)]
    for i in range(4):
        nc.vector.memset(ve_tiles[i][:, :, DH:DH + 1], 1.0)

    # ---- qkv projection, split into 6 units per pair so units can be
    # software-pipelined into the previous pair's key-tile loop ----
    qkv_tiles = {}

    def qkv_unit(p, u):
        name, m = (("q", p), ("k", NPAIR + p), ("v", 2 * NPAIR + p))[u // 2]
        qc = u % 2
        if qc == 0:
            qkv_tiles[(p, name)] = qkv_pool.tile(
                [P, N], BF16, tag=name, name=f"{name}{p}")
        dst = qkv_tiles[(p, name)]
        ps = ps_pool.tile([P, 512], F32, tag="ps", name=f"ps_{name}{p}_{qc}")
        for k in range(KT):
            nc.tensor.matmul(
                out=ps[:],
                lhsT=wqkv_b[:, k, m * P:(m + 1) * P],
                rhs=xT[:, k, qc * 512:(qc + 1) * 512],
                start=(k == 0), stop=(k == KT - 1))
        nc.vector.tensor_copy(out=dst[:, qc * 512:(qc + 1) * 512], in_=ps[:])

    for u in range(6):
        qkv_unit(0, u)

    # ---- per head pair ----
    for p in range(NPAIR):
        qt = qkv_tiles[(p, "q")]
        kt_ = qkv_tiles[(p, "k")]
        vt = qkv_tiles[(p, "v")]

        av = [ps_av.tile([DH + 1, N], F32, tag="ps_av", name=f"av{p}_{hi}")
              for hi in range(2)]

        def pv_mms(ve_, pts_, kb_):
            for hi in range(2):
                for qc in range(2):
                    nc.tensor.matmul(
                        out=av[hi][:, qc * 512:(qc + 1) * 512],
                        lhsT=ve_[:, hi, :],
                        rhs=pts_[qc][:, 512 * hi:512 * (hi + 1)],
                        start=(kb_ == 0), stop=(kb_ == NT - 1))

        prev = None
        for kb in range(NT):
            # v natural block for this key tile
            vnat = ps_pool.tile([P, P], BF16, tag="ps", name=f"vn{p}_{kb}")
            nc.tensor.transpose(
                out=vnat[:], in_=vt[:, kb * P:(kb + 1) * P], identity=ident[:])
            ve = ve_tiles[kb % 4]
            nc.vector.tensor_copy(
                out=ve[:, :, 0:DH], in_=vnat[:].rearrange("p (h d) -> p h d", h=2))

            # both heads' score matmuls target one psum tile (h0 in the low
            # bank, h1 in the high bank) so they are consecutive on the PE
            # queue and stream through disjoint row groups concurrently;
            # the mask bias is per-key so one exp covers both heads
            pts = []
            for qc in range(2):
                psq = ps_pool.tile([P, N], F32, tag="ps", name=f"ps_s{p}_{kb}_{qc}")
                for hi in range(2):
                    nc.tensor.matmul(
                        out=psq[:, 512 * hi:512 * (hi + 1)],
                        lhsT=kt_[64 * hi:64 * (hi + 1), kb * P:(kb + 1) * P],
                        rhs=qt[64 * hi:64 * (hi + 1), qc * 512:(qc + 1) * 512],
                        start=True, stop=True)
                pt = pt_pool.tile([P, N], BF16, tag="pt", name=f"pt{p}_{kb}_{qc}")
                nc.scalar.activation(
                    out=pt[:], in_=psq[:], func=AF.Exp,
                    bias=maskb[:, kb:kb + 1], scale=SCALE)
                pts.append(pt)

            # p@v for the PREVIOUS key tile: its exps finished long ago, so
            # the in-order PE queue never waits on ACT here
            if prev is not None:
                pv_mms(*prev)

            # PE filler while this tile's exps run: next pair's qkv units
            if p + 1 < NPAIR and kb < 6:
                qkv_unit(p + 1, kb)

            prev = (ve, pts, kb)
        pv_mms(*prev)

        for hi in range(2):
            # copy out of PSUM promptly so the av slot frees for pair p+1
            av_sb = zb_pool.tile([DH + 1, N], F32, tag="av_sb", name=f"avs{p}_{hi}")
            nc.scalar.activation(out=av_sb[:], in_=av[hi][:], func=AF.Copy)
            zdram = dram_pool.tile([1, N], F32, tag="zdram", name=f"zd{p}_{hi}")
            nc.sync.dma_start(out=zdram[:], in_=av_sb[DH:DH + 1, :])
            zb = zb_pool.tile([DH, N], F32, tag="zb", name=f"zb{p}_{hi}")
            nc.sync.dma_start(out=zb[:], in_=zdram[0:1, :].to_broadcast([DH, N]))
            nc.vector.reciprocal_approx_fast(out=zb[:], in_=zb[:])
            nc.vector.scalar_tensor_tensor(
                out=attnT[64 * hi:64 * (hi + 1), p, :],
                in0=av_sb[0:DH, :], scalar=1.0, in1=zb[:],
                op0=ALU.mult, op1=ALU.mult)

    # ---- projection back to natural layout ----
    # psum-bank-aligned output chunks: [0:512) and [512:768)
    chunks = [(0, 512), (512, 256)]
    for m in range(NT):
        out_sb = xnat_pool.tile([P, C], F32, tag="out_sb", name=f"out_sb{m}")
        pps = [ps_pool.tile([P, w], F32, tag="ps", name=f"ps_proj{m}_{j}")
               for j, (lo, w) in enumerate(chunks)]
        for j, (lo, w) in enumerate(chunks):
            for k in range(KT):
                nc.tensor.matmul(
                    out=pps[j][:],
                    lhsT=attnT[:, k, m * P:(m + 1) * P],
                    rhs=wproj_sb[:, k, lo:lo + w],
                    start=(k == 0), stop=(k == KT - 1))
            nc.vector.scalar_tensor_tensor(
                out=out_sb[:, lo:lo + w], in0=pps[j][:], scalar=1.0,
                in1=bias_bc[:, lo:lo + w], op0=ALU.mult, op1=ALU.add)
        nc.sync.dma_start(out=out_ext[m * P:(m + 1) * P, :], in_=out_sb[:])


def build():
    nc = bacc.Bacc()
    x_ext = nc.declare_dram_parameter("x", [N, C], F32, isOutput=False)
    mask_ext = nc.declare_dram_parameter("mask", [NT, P], I32, isOutput=False)
    wqkv_ext = nc.declare_dram_parameter("w_qkv", [C, 3 * C], F32, isOutput=False)
    wproj_ext = nc.declare_dram_parameter("w_proj", [C, C], F32, isOutput=False)
    bproj_ext = nc.declare_dram_parameter("b_proj", [1, C], F32, isOutput=False)
    out_ext = nc.declare_dram_parameter("out", [N, C], F32, isOutput=True)

    with tile.TileContext(nc) as tc, ExitStack() as ctx:
        _body(ctx, tc, x_ext.ap(), mask_ext.ap(), wqkv_ext.ap(), wproj_ext.ap(),
              bproj_ext.ap(), out_ext.ap())
    nc.finalize()
    return nc


_NC_CACHE = None


def _get_nc():
    global _NC_CACHE
    if _NC_CACHE is None:
        _NC_CACHE = build()
    return _NC_CACHE


def _make_in_maps(inputs):
    x = np.ascontiguousarray(np.asarray(inputs["x"], dtype=np.float32))
    mask = np.ascontiguousarray(np.asarray(inputs["mask"], dtype=np.int32))
    w_qkv = np.ascontiguousarray(np.asarray(inputs["w_qkv"], dtype=np.float32))
    w_proj = np.ascontiguousarray(np.asarray(inputs["w_proj"], dtype=np.float32))
    b_proj = np.ascontiguousarray(
        np.asarray(inputs["b_proj"], dtype=np.float32)).reshape(1, C)
    return [
        {
            "x": x[b],
            "mask": mask[b].reshape(NT, P),
            "w_qkv": w_qkv,
            "w_proj": w_proj,
            "b_proj": b_proj,
        }
        for b in range(B)
    ]


def _run(inputs, trace=False, **kwargs):
    nc = _get_nc()
    in_maps = _make_in_maps(inputs)
    res = run_bass_kernel_spmd(nc, in_maps, list(range(NCORES)), trace=trace, **kwargs)
    out = np.stack([np.asarray(res.results[i]["out"]) for i in range(NCORES)])
    return out, res


def kernel(**inputs):
    out, _ = _run(inputs)
    return out
